# revision 27
# baseline (speedup 1.0000x reference)
"""Causal self-attention kernel for Trainium2, 8 NeuronCores.

Problem: B=4, T=2048, C=1024, 16 heads, D=64 (fp32).
Sharding: core i handles batch b=i//2 and head-group hg=i%2 (8 heads each).
Each core computes qkv + attention + its partial projection; the host sums
the two head-group partials per batch and adds b_proj.

Single software-pipelined stream per core (weights fp32r, activations bf16
in the attention interior; all matmuls 1 PE-cycle/row):
  - x tiles DMA'd on 3 queues; per tile: PE-transpose (bf16 identity) ->
    XT chunks, V = x @ Wv -> VA (bf16, ones-augmented per head).
  - g0's QKT chunks and attention strips interleave with the x pipeline as
    their tiles become ready.
  - Attention per head-pair g: scores S^T via row-packed K=64 matmuls, exp
    on Act (bf16 out), causal triangle masks on gpsimd, PV accumulation
    with ones-augmented V; scores(s+1) is emitted before PV(s) so the PE
    never head-of-line blocks on the exp.
  - QKT chunks for head-pair g+1 (and proj tiles during g=3) are emitted as
    PE filler inside the Act-bound strip loop.
  - Normalization via reciprocal + K=1 broadcast matmul; odd head partition-
    shifted into pair-stacked OT via SBUF->SBUF DMA.
  - proj: out = OT^T @ Wproj partial, per t-tile, DMA'd out.
"""

import numpy as np

N_CORES = 8
T = 2048
C = 1024
HL = 8          # heads per core
D = 64
KC = C // 128   # 8 contraction chunks
NT = T // 128   # 16 t-tiles
NQ = T // 512   # 4 q-tiles
VW = HL * 65    # 520 v-aug cols per t-tile

_CACHE = {}


def _build():
    from contextlib import ExitStack
    import concourse.bass as bass
    from concourse import bacc
    import concourse.mybir as mybir
    import concourse.tile as tile
    from concourse.masks import make_identity

    F32 = mybir.dt.float32
    F32R = mybir.dt.float32r
    BF16 = mybir.dt.bfloat16
    EXP = mybir.ActivationFunctionType.Exp
    ISGE = mybir.AluOpType.is_ge
    W15 = C + C // 2  # 1536

    nc = bacc.Bacc("TRN2", target_bir_lowering=False, debug=False,
                   num_devices=N_CORES)

    x_d = nc.dram_tensor("x", [T, C], F32, kind="ExternalInput")
    wqkv_d = nc.dram_tensor("w_qkv", [C, W15], F32, kind="ExternalInput")
    wproj_d = nc.dram_tensor("w_proj", [512, C], F32, kind="ExternalInput")
    bqk_d = nc.dram_tensor("b_qk", [128, 8], F32, kind="ExternalInput")
    bv_d = nc.dram_tensor("b_v", [128, 512], F32, kind="ExternalInput")
    ones_d = nc.dram_tensor("ones64", [128, 128], F32, kind="ExternalInput")
    ident_d = nc.dram_tensor("ident128", [128, 128], F32, kind="ExternalInput")
    out_d = nc.dram_tensor("out", [T, C], F32, kind="ExternalOutput")

    # computed q-range per diagonal delta; causally-dead region skipped,
    # each delta's [128]-wide triangle block gets an affine_select.
    QOFF = (0, 128, 256, 384)

    with tile.TileContext(nc) as tc, ExitStack() as ctx:
        # ---------- persistent pools ----------
        consts = ctx.enter_context(tc.tile_pool(name="consts", bufs=1))
        big = ctx.enter_context(tc.tile_pool(name="big", bufs=1))
        psmm = ctx.enter_context(tc.tile_pool(name="psmm", bufs=2, space="PSUM"))
        psS = ctx.enter_context(tc.tile_pool(name="psS", bufs=2, space="PSUM"))
        psO = ctx.enter_context(tc.tile_pool(name="psO", bufs=1, space="PSUM"))
        wqkp = ctx.enter_context(tc.tile_pool(name="wqk", bufs=2))
        qktp = ctx.enter_context(tc.tile_pool(name="qkt", bufs=2))
        ptp = ctx.enter_context(tc.tile_pool(name="ptile", bufs=3))
        rscp = ctx.enter_context(tc.tile_pool(name="rsc", bufs=2))

        identr_t = consts.tile([128, 128], F32R)
        nc.gpsimd.dma_start(out=identr_t[:], in_=ident_d[:].bitcast(F32R))
        identr = identr_t[:]
        bqk_sb = consts.tile([128, 8], F32)
        nc.gpsimd.dma_start(out=bqk_sb[:], in_=bqk_d[:])
        bv_sb = consts.tile([128, 512], F32)
        nc.gpsimd.dma_start(out=bv_sb[:], in_=bv_d[:])
        ones_sb = consts.tile([128, 128], F32R)
        nc.gpsimd.dma_start(out=ones_sb[:], in_=ones_d[:].bitcast(F32R))

        XT = big.tile([128, KC * T], F32R)        # 64 KB/part, x transposed
        VA = big.tile([128, NT * VW], BF16)       # 16.25 KB/part, v-aug
        OT = big.tile([128, 4 * T], F32R)         # 32 KB/part, attn out^T

        # ---------------- emission helpers ----------------
        fillq = []          # PE filler thunks popped inside strip loops
        fill_state = {"n": 0, "done": 0, "total": 0, "strips": 1}

        def set_fill(q, strips):
            nonlocal fillq
            fillq = q
            fill_state.update(n=0, done=0, total=len(q) or 16, strips=strips)

        def fill_tick():
            # proportional pacing: spread the filler supply over the strip
            # count so the Act-bound stretches all get PE work
            fill_state["n"] += 1
            want = -(-fill_state["n"] * fill_state["total"]
                     // fill_state["strips"])
            while fill_state["done"] < want and fillq:
                item = fillq.pop(0)
                (item[1] if isinstance(item, tuple) else item)()
                fill_state["done"] += 1

        wqk_t = {}
        qkt_t = {}
        qkq = {g: [] for g in range(4)}   # pending (nt4, thunk) per g

        def load_wqk(g, queues):
            wqk = wqkp.tile([128, 2 * KC * 128], F32R, tag="wqk", name="wqk")
            for half in (0, 1):
                queues[half].dma_start(
                    out=wqk[:, half * KC * 128:(half + 1) * KC * 128]
                        .rearrange("p (k m) -> p k m", k=KC),
                    in_=wqkv_d[:, half * 512 + g * 128: half * 512 + (g + 1) * 128]
                        .rearrange("(k p) m -> p k m", p=128).bitcast(F32R),
                )
            wqk_t[g] = wqk
            qkt_t[g] = qktp.tile([128, 2 * T], BF16, tag="qkt", name="qkt")
            for nt4 in range(NQ):
                for half in (0, 1):
                    qkq[g].append((nt4, _mk_qk(g, half, nt4)))

        def _mk_qk(g, half, nt4):
            def thunk():
                wqk = wqk_t[g]
                qkt = qkt_t[g]
                pqk = psmm.tile([128, 512], F32, tag="mm", name="pqk")
                for k in range(KC):
                    nc.tensor.matmul(
                        pqk[:],
                        wqk[:, half * KC * 128 + k * 128:
                               half * KC * 128 + (k + 1) * 128],
                        XT[:, k * T + nt4 * 512: k * T + (nt4 + 1) * 512],
                        start=(k == 0), stop=(k == KC - 1))
                nc.vector.tensor_scalar_add(
                    qkt[:, half * T + nt4 * 512: half * T + (nt4 + 1) * 512],
                    pqk[:],
                    bqk_sb[:, half * 4 + g: half * 4 + g + 1])
            return thunk

        def ensure_qk(g, qt):
            while qkq[g] and qkq[g][0][0] <= qt:
                qkq[g].pop(0)[1]()

        def emit_strips(g, qt, norm_prev=None):
            qkt = qkt_t[g]
            psO0 = psO.tile([65, 512], F32, tag="o0", name="psO0")
            psO1 = psO.tile([65, 512], F32, tag="o1", name="psO1")
            psOh = [psO0, psO1]
            jlast = 4 * qt + 3
            nstrips = 2 * qt + 2

            def emit_pv(s, pts, diag):
                for hi in (0, 1):
                    h = 2 * g + hi
                    for dd in (0, 1):
                        j = 2 * s + dd
                        qoff = QOFF[j - 4 * qt] if diag else 0
                        nc.tensor.matmul(
                            psOh[hi][0:65, qoff:512],
                            VA[:, j * VW + h * 65: j * VW + (h + 1) * 65],
                            pts[hi][:, dd * 512 + qoff:(dd + 1) * 512],
                            start=(j == 0), stop=(j == jlast))

            pending = None
            for s in range(nstrips):
                diag = s >= 2 * qt
                pts = []
                for hi in (0, 1):
                    psSt = psS.tile([128, 1024], F32, tag="psS", name="psSt")
                    for dd in (0, 1):
                        j = 2 * s + dd
                        qoff = QOFF[j - 4 * qt] if diag else 0
                        nc.tensor.matmul(
                            psSt[:, dd * 512 + qoff:(dd + 1) * 512],
                            qkt[64 * hi:64 * hi + 64,
                                T + j * 128: T + (j + 1) * 128],
                            qkt[64 * hi:64 * hi + 64,
                                qt * 512 + qoff:(qt + 1) * 512],
                            start=True, stop=True,
                            tile_position=(64 * hi, 0))
                    ptile = ptp.tile([128, 1024], BF16, tag=f"pt{hi}",
                                     name="ptile")
                    if diag and s == 2 * qt + 1:
                        # deltas 2,3: only cols [256:512] / [896:1024] computed
                        nc.scalar.activation(
                            ptile[:, 256:512], psSt[:, 256:512],
                            EXP, scale=0.125)
                        nc.scalar.activation(
                            ptile[:, 896:1024], psSt[:, 896:1024],
                            EXP, scale=0.125)
                    else:
                        nc.scalar.activation(ptile[:], psSt[:], EXP,
                                             scale=0.125)
                    if diag:
                        for dd in (0, 1):
                            delta = 2 * (s - 2 * qt) + dd
                            # triangle block: keep where (q rel block) - k >= 0
                            sl = slice(dd * 512 + 128 * delta,
                                       dd * 512 + 128 * delta + 128)
                            nc.gpsimd.affine_select(
                                out=ptile[:, sl], in_=ptile[:, sl],
                                compare_op=ISGE, fill=0.0, base=0,
                                pattern=[[1, 128]],
                                channel_multiplier=-1)
                    pts.append(ptile)
                if s == 0 and norm_prev is not None:
                    norm_prev()
                fill_tick()
                if pending is not None:
                    emit_pv(*pending)
                pending = (s, pts, diag)
            emit_pv(*pending)

            def norm_thunk():
                # normalize + store OT (deferred into the next strip loop so
                # the PE stays warm over the reciprocal latency)
                r = rscp.tile([128, 1024], F32R, tag="r", name="r")
                with nc.allow_low_precision(reason="softmax reciprocal"):
                    for hi in (0, 1):
                        nc.vector.reciprocal(
                            r[64:65, hi * 512:(hi + 1) * 512],
                            psOh[hi][64:65, :])
                for hi in (0, 1):
                    bc = psmm.tile([64, 512], F32, tag="mm", name="bc")
                    nc.tensor.matmul(
                        bc[:], ones_sb[64:65, 0:64],
                        r[64:65, hi * 512:(hi + 1) * 512],
                        start=True, stop=True)
                    bc_sb = rscp.tile([64, 512], BF16, tag="bcsb",
                                      name="bc_sb")
                    nc.vector.tensor_copy(bc_sb[:], bc[:])
                    if hi == 0:
                        nc.vector.tensor_mul(
                            OT[0:64, g * T + qt * 512: g * T + (qt + 1) * 512],
                            psOh[0][0:64, :], bc_sb[:])
                    else:
                        otmp = rscp.tile([64, 512], F32R, tag="otmp",
                                         name="otmp")
                        nc.vector.tensor_mul(otmp[:], psOh[1][0:64, :],
                                             bc_sb[:])
                        nc.sync.dma_start(
                            out=OT[64:128,
                                   g * T + qt * 512: g * T + (qt + 1) * 512],
                            in_=otmp[:])
                if g == 3:
                    for it in range(4 * qt, 4 * qt + 4):
                        projq.append(_mk_proj(it))
            return norm_thunk

        # ---------------- fused x-pipeline + attention ----------------
        with (
            tc.tile_pool(name="xnat", bufs=4) as xnat,
            tc.tile_pool(name="wv", bufs=1) as wvp,
        ):
            DQ = (nc.sync, nc.scalar, nc.gpsimd)
            wv = wvp.tile([128, KC * 512], F32R)
            xts = [xnat.tile([128, C], F32R, tag="xn", name="xt_t")
                   for _ in range(NT)]

            def dma_x(it, q):
                q.dma_start(out=xts[it][:],
                            in_=x_d[it * 128:(it + 1) * 128, :].bitcast(F32R))

            def dma_wv(half, q):
                q.dma_start(
                    out=wv[:].rearrange("p (k m) -> p k m", k=KC)
                        [:, half * 4:(half + 1) * 4, :],
                    in_=wqkv_d[half * 512:(half + 1) * 512, 1024:1536]
                        .rearrange("(k p) m -> p k m", p=128).bitcast(F32R),
                )

            # head: interleave wv with the first x tiles so transposes and
            # V(0) both start early
            dma_wv(0, nc.sync)
            dma_x(0, nc.scalar)
            dma_x(1, nc.gpsimd)
            dma_wv(1, nc.sync)
            dma_x(2, nc.scalar)
            dma_x(3, nc.gpsimd)
            load_wqk(0, (nc.sync, nc.scalar))
            for it in range(4, NT):
                dma_x(it, (nc.gpsimd, nc.scalar, nc.sync)[it % 3])

            def emit_ab(it4):
                for j in range(4):
                    it = it4 * 4 + j
                    pt = psS.tile([128, 1024], F32R, tag="psS", name="pt")
                    for c in range(KC):
                        nc.tensor.transpose(
                            pt[:, c * 128:(c + 1) * 128],
                            xts[it][:, c * 128:(c + 1) * 128],
                            identr)
                    nc.vector.tensor_copy(
                        XT[:].rearrange("p (k t) -> p k t", k=KC)
                            [:, :, it * 128:(it + 1) * 128],
                        pt[:].rearrange("p (k t) -> p k t", k=KC))
                for j in range(4):
                    it = it4 * 4 + j
                    pv = psmm.tile([128, 512], F32, tag="mm", name="pv")
                    for k in range(KC):
                        nc.tensor.matmul(
                            pv[:],
                            XT[:, k * T + it * 128: k * T + (it + 1) * 128],
                            wv[:, k * 512:(k + 1) * 512],
                            start=(k == 0), stop=(k == KC - 1))
                    va_dst = VA[:, it * VW:(it + 1) * VW].rearrange(
                        "p (h c) -> p h c", h=HL)[:, :, 0:64]
                    nc.vector.tensor_add(
                        va_dst,
                        pv[:].rearrange("p (h c) -> p h c", h=HL),
                        bv_sb[:].rearrange("p (h c) -> p h c", h=HL))
                    # ones column of each 65-wide v-aug group for this tile
                    nc.vector.tensor_copy(
                        VA[:, it * VW + 64:(it + 1) * VW:65],
                        ones_sb[:, 0:HL].bitcast(F32))

            # interleave g0's qk/strips with the x pipeline
            norm_prev = None
            for qt in range(NQ):
                emit_ab(qt)
                ensure_qk(0, qt)
                if qt == 2:
                    load_wqk(1, (nc.sync, nc.gpsimd))
                if qt < NQ - 1:
                    norm_prev = emit_strips(0, qt, norm_prev)
            # last qt of g0 emitted after the x pools close

        # proj weights + staging go where xnat/wv lived
        wpp = ctx.enter_context(tc.tile_pool(name="wp", bufs=1))
        stagep = ctx.enter_context(tc.tile_pool(name="stage", bufs=2))
        wp = wpp.tile([128, 4 * C], F32R)
        nc.sync.dma_start(
            out=wp[:].rearrange("p (g m) -> p g m", g=4),
            in_=wproj_d[:].rearrange("(g p) m -> p g m", p=128).bitcast(F32R),
        )

        set_fill(qkq[1], 8)  # g1's chunks fill g0's 8 last-qt strips
        norm_prev = emit_strips(0, NQ - 1, norm_prev)

        def _mk_proj(it):
            def thunk():
                stage = stagep.tile([128, C], F32, tag="stg", name="stage")
                for n in (0, 1):
                    pp = psmm.tile([128, 512], F32, tag="mm", name="pp")
                    for gg in range(4):
                        nc.tensor.matmul(
                            pp[:],
                            OT[:, gg * T + it * 128: gg * T + (it + 1) * 128],
                            wp[:, gg * C + n * 512: gg * C + (n + 1) * 512],
                            start=(gg == 0), stop=(gg == 3))
                    nc.vector.tensor_copy(stage[:, n * 512:(n + 1) * 512],
                                          pp[:])
                    (nc.sync, nc.gpsimd)[(2 * it + n) % 2].dma_start(
                        out=out_d[it * 128:(it + 1) * 128,
                                  n * 512:(n + 1) * 512],
                        in_=stage[:, n * 512:(n + 1) * 512])
            return thunk

        projq = []
        for g in range(1, 4):
            if g < 3:
                load_wqk(g + 1, (nc.sync, nc.gpsimd))
                set_fill(qkq[g + 1], 20)
            else:
                set_fill(projq, 20)
            for qt in range(NQ):
                ensure_qk(g, qt)
                norm_prev = emit_strips(g, qt, norm_prev)
        norm_prev()
        while projq:
            projq.pop(0)()

    nc.compile()
    return nc


def _in_maps(x, W_attn, b_attn, W_proj, b_proj):
    ones64 = np.ones((128, 128), np.float32)

    in_maps = []
    for core in range(N_CORES):
        b = core // 2
        hg = core % 2
        sl = slice(hg * 512, (hg + 1) * 512)
        w_qkv = np.concatenate(
            [W_attn[:, 0:1024][:, sl], W_attn[:, 1024:2048][:, sl],
             W_attn[:, 2048:3072][:, sl]], axis=1)
        bq = b_attn[0:1024][sl]
        bk = b_attn[1024:2048][sl]
        bv = b_attn[2048:3072][sl]
        # b_qk [128, 8]: col half*4+g holds bias for W cols (half,g) chunk
        b_qk = np.stack(
            [bq[g * 128:(g + 1) * 128] for g in range(4)]
            + [bk[g * 128:(g + 1) * 128] for g in range(4)], axis=1)
        b_v = np.broadcast_to(bv, (128, 512)).copy()
        in_maps.append({
            "x": np.ascontiguousarray(x[b]),
            "w_qkv": np.ascontiguousarray(w_qkv),
            "w_proj": np.ascontiguousarray(W_proj[sl, :]),
            "b_qk": np.ascontiguousarray(b_qk.astype(np.float32)),
            "b_v": b_v.astype(np.float32),
            "ones64": ones64,
            "ident128": np.eye(128, dtype=np.float32),
        })
    return in_maps


def kernel(x, W_attn, b_attn, W_proj, b_proj, _trace=False):
    from concourse.bass_utils import run_bass_kernel_spmd

    x = np.asarray(x, dtype=np.float32)
    W_attn = np.asarray(W_attn, dtype=np.float32)
    b_attn = np.asarray(b_attn, dtype=np.float32)
    W_proj = np.asarray(W_proj, dtype=np.float32)
    b_proj = np.asarray(b_proj, dtype=np.float32)

    if "nc" not in _CACHE:
        _CACHE["nc"] = _build()
    nc = _CACHE["nc"]

    in_maps = _in_maps(x, W_attn, b_attn, W_proj, b_proj)
    res = run_bass_kernel_spmd(nc, in_maps, list(range(N_CORES)), trace=_trace)
    B = x.shape[0]
    out = np.empty((B, T, C), np.float32)
    for b in range(B):
        out[b] = res.results[2 * b]["out"] + res.results[2 * b + 1]["out"] + b_proj
    if _trace:
        _CACHE["last_result"] = res
    return out
